# revision 6
# baseline (speedup 1.0000x reference)
"""Connectome kernel (segment-mean -> Pearson Gram) for 8 TRN2 NeuronCores.

Strategy (pure data parallel, 2 samples per core):
  - Host marshalling: fold mask into parcellation; DROP background /
    masked-out pixels (~50% of V) entirely; sort surviving pixels by ROI
    and pack them into 128-pixel chunks (block B = ROIs 128..199 FIRST,
    then block A = ROIs 0..127; each block padded to a chunk boundary
    with label -1 slots). x is gathered into this packed order, cast
    fp16, laid out [p, chunk, sample, t] per core so each SBUF partition
    reads one contiguous HBM run per chunk-tile. Wire traffic per core:
    ~18.3MB (vs 73.7MB for fp32 all-pixels).
  - Device: stream chunk-tiles on the two HWDGE rings (consts go on the
    gpsimd SWDGE ring so both HWDGE rings start on x immediately);
    onehots for each tile are built in batched DVE tensor_tensor ops
    (is_equal of broadcast iota vs broadcast labels); per chunk one PE
    matmul acc[r, row] += onehot.T @ x_chunk (fp16 operands, fp32 PSUM).
  - Centering cancels analytically: C C^T = S S^T - T m m^T, so the
    device Grams the RAW per-ROI sums S and ships per-(roi, sample) row
    sums (tiny); the host applies the rank-1 correction and the 1/norm
    scaling (norms come from the corrected diagonal). The ROI-mean /c
    and the +eps normalizer cancel in the Pearson Gram.
  - Per block: cast PSUM sums to fp16 (one DVE copy), reduce row sums,
    PE-transpose into [t, roi] tiles. Block B's work overlaps block A's
    stream; block A + Gram + fp16 conn output form a short tail.
  - Host: concat cores, rank-1 correct, normalize, upper triangle
    -> (16, 19900) fp32.
"""
import sys

sys.path.insert(0, "/opt/trn_rl_repo")

import numpy as np

import concourse.bass as bass
import concourse.tile as tile
from concourse import bacc, mybir
from concourse.bass_utils import run_bass_kernel_spmd

F32 = mybir.dt.float32
F16 = mybir.dt.float16

N, T, H, W = 16, 200, 144, 320
V = H * W                      # 46080
R = 200                        # ROIs
RA = 128                       # ROI block A width (ROIs 0..127)
RB = R - RA                    # ROI block B width (72; ROIs 128..199)
NCORES = 8
SPB = N // NCORES              # samples per core = 2
ROWS = SPB * T                 # 400
EPS = 1e-8


def _tile_sizes(nch):
    """DMA tile schedule: small first tiles to fill the pipe fast, big
    16-chunk tiles in steady state (fewer DMAs -> fewer semaphores and
    issue slots), small tapered tiles at the end so the PE drain after
    the last transfer is short."""
    sizes, left = [], nch
    while left and len(sizes) < 4:
        ct = min(4, left)
        sizes.append(ct)
        left -= ct
    while left >= 24:
        sizes.append(16)
        left -= 16
    while left >= 12:
        sizes.append(8)
        left -= 8
    if left > 8:
        sizes.append(left - 8)
        left = 8
    while left:
        ct = min(4, left)
        sizes.append(ct)
        left -= ct
    return sizes


_cached = {}


def _bc3(ap2, ins_pos, n):
    """Insert a broadcast (stride 0, count n) dim into a 2D AP."""
    layout = [list(d) for d in ap2.ap]
    layout.insert(ins_pos, [0, n])
    return bass.AP(ap2.tensor, ap2.offset, layout)


def _split_st(ap2):
    """View a [P, SPB*T] AP as [P, SPB, T] (split the free dim)."""
    layout = [list(d) for d in ap2.ap]
    assert layout[-1][0] == 1 and layout[-1][1] == SPB * T
    layout = layout[:-1] + [[T, SPB], [1, T]]
    return bass.AP(ap2.tensor, ap2.offset, layout)


def _build_program(nA, nB):
    nch = nA + nB
    nc = bacc.Bacc("TRN2", target_bir_lowering=False, debug=False)

    # consts packed into one DRAM tensor: cols [0:nch] labs, [nch:+128]
    # iota, [+128:+256] i128, [+256:+328] i72 (partitions 72:128 zero).
    CC = nch + 328
    x_d = nc.declare_dram_parameter("x", [128, nch, ROWS], F16, isOutput=False)
    cst_d = nc.declare_dram_parameter("consts", [128, CC], F16, isOutput=False)
    out_d = nc.declare_dram_parameter("conn", [SPB, R, R], F16, isOutput=True)
    msa_d = nc.declare_dram_parameter("msa", [RA, SPB], F32, isOutput=True)
    msb_d = nc.declare_dram_parameter("msb", [RB, SPB], F32, isOutput=True)

    tsizes = _tile_sizes(nch)

    with tile.TileContext(nc) as tc:
        with tc.tile_pool(name="consts", bufs=1) as consts, \
             tc.tile_pool(name="loads", bufs=3) as loads, \
             tc.tile_pool(name="ohp", bufs=1) as ohp, \
             tc.tile_pool(name="epi", bufs=1) as epi, \
             tc.tile_pool(name="psum", bufs=1, space="PSUM") as psum:

            cst_s = consts.tile([128, CC], F16)
            # one consts DMA on the SWDGE (gpsimd) ring: both HWDGE rings
            # start streaming x immediately.
            nc.gpsimd.dma_start(cst_s[:], cst_d[:])
            labs_s = cst_s[:, 0:nch]
            iota_s = cst_s[:, nch:nch + 128]
            i128_s = cst_s[:, nch + 128:nch + 256]
            i72_s = cst_s[0:72, nch + 256:nch + 328]

            acc_a = psum.tile([RA, ROWS], F32, tag="acc_a", bufs=1)
            acc_b = psum.tile([RB, ROWS], F32, tag="acc_b", bufs=1)

            # PSUM tr tiles: [t-block, roi] transposed raw-sum rows.
            tr = {}
            for s in range(SPB):
                tr[("A", s)] = psum.tile([128, R], F16, tag="trA", bufs=2,
                                         name=f"trA_{s}")
                tr[("B", s)] = psum.tile([72, R], F16, tag="trB", bufs=2,
                                         name=f"trB_{s}")

            def finish_block(blk, acc, P, ms_d):
                """Raw-sum epilogue for one ROI block: cast the PSUM sums
                to fp16 (Gram/transpose operand) and ship per-sample row
                sums (host applies the rank-1 centering correction)."""
                S16 = epi.tile([P, ROWS], F16, tag=f"S16_{blk}")
                ms = epi.tile([P, SPB], F32, tag=f"ms_{blk}")
                nc.vector.tensor_copy(S16[:], acc[:])
                nc.vector.tensor_reduce(ms[:], _split_st(acc[:]),
                                        axis=mybir.AxisListType.X,
                                        op=mybir.AluOpType.add)
                nc.gpsimd.dma_start(ms_d[:], ms[:])
                return S16

            S16_b = None
            with nc.named_scope("main"):
                ch0 = 0
                for ti, ct in enumerate(tsizes):
                    ld = loads.tile([128, ct, ROWS], F16, tag=f"ld{ct}",
                                    bufs=4, name=f"ld_{ti}")
                    eng = nc.scalar if (ti % 2 == 0) else nc.sync
                    eng.dma_start(ld[:], x_d[:, ch0:ch0 + ct, :])

                    # batched per-tile onehot builds (DVE), one per block
                    # segment present in this tile
                    nb_i = max(0, min(nB, ch0 + ct) - ch0)       # B chunks
                    na_i = ct - nb_i                             # A chunks
                    ohB_t = ohA_t = None
                    if nb_i:
                        ohB_t = ohp.tile([128, nb_i, RB], F16,
                                         tag=f"ohB{nb_i}", bufs=3,
                                         name=f"ohB_{ti}")
                        nc.vector.tensor_tensor(
                            ohB_t[:], _bc3(iota_s[:, 0:RB], 1, nb_i),
                            _bc3(labs_s[:, ch0:ch0 + nb_i], 2, RB),
                            op=mybir.AluOpType.is_equal)
                    if na_i:
                        a0 = ch0 + nb_i
                        ohA_t = ohp.tile([128, na_i, RA], F16,
                                         tag=f"ohA{na_i}", bufs=3,
                                         name=f"ohA_{ti}")
                        nc.vector.tensor_tensor(
                            ohA_t[:], _bc3(iota_s[:, 0:RA], 1, na_i),
                            _bc3(labs_s[:, a0:a0 + na_i], 2, RA),
                            op=mybir.AluOpType.is_equal)

                    for j in range(ct):
                        cc = ch0 + j
                        if cc < nB:
                            acc, oh = acc_b, ohB_t[:, j, :]
                            start, stop = (cc == 0), (cc == nB - 1)
                        else:
                            acc, oh = acc_a, ohA_t[:, j - nb_i, :]
                            start, stop = (cc == nB), (cc == nch - 1)
                        nc.tensor.matmul(acc[:], oh, ld[:, j, :],
                                         start=start, stop=stop)
                    ch0 += ct

                    if ch0 - ct < nB <= ch0:
                        # block B complete: cast + row sums on DVE while
                        # block A still streams.
                        b_done_ti = ti
                        S16_b = finish_block("b", acc_b, RB, msb_d)
                    if S16_b is not None and ti == b_done_ti + 2:
                        # B-sourced transposes, emitted a couple of tiles
                        # later so the cast has finished and PE's FIFO
                        # never blocks on it.
                        for s in range(SPB):
                            nc.tensor.transpose(
                                tr[("A", s)][:, 128:200],
                                S16_b[:, s * T:s * T + 128], i72_s)
                            nc.tensor.transpose(
                                tr[("B", s)][:, 128:200],
                                S16_b[:, s * T + 128:s * T + 200], i72_s)

            with nc.named_scope("epilogue"):
                S16_a = finish_block("a", acc_a, RA, msa_d)
                for s in range(SPB):
                    trA, trB = tr[("A", s)], tr[("B", s)]
                    nc.tensor.transpose(trA[:, 0:128],
                                        S16_a[:, s * T:s * T + 128],
                                        i128_s)
                    nc.tensor.transpose(trB[:, 0:128],
                                        S16_a[:, s * T + 128:s * T + 200],
                                        i128_s)
                    trA_sb = epi.tile([128, R], F16, name=f"trAs_{s}",
                                      tag="trAs", bufs=2)
                    trB_sb = epi.tile([72, R], F16, name=f"trBs_{s}",
                                      tag="trBs", bufs=2)
                    nc.vector.tensor_copy(trA_sb[:], trA[:])
                    nc.vector.tensor_copy(trB_sb[:], trB[:])

                    # Gram: conn = S_t.T @ S_t  (contraction over t, fp16)
                    cA = psum.tile([128, R], F32, tag="cA", bufs=1,
                                   name=f"cA_{s}")
                    cB = psum.tile([72, R], F32, tag="cB", bufs=1,
                                   name=f"cB_{s}")
                    nc.tensor.matmul(cA[:], trA_sb[:, 0:128], trA_sb[:],
                                     start=True, stop=False)
                    nc.tensor.matmul(cA[:], trB_sb[:, 0:128], trB_sb[:],
                                     start=False, stop=True)
                    nc.tensor.matmul(cB[:], trA_sb[:, 128:200], trA_sb[:],
                                     start=True, stop=False)
                    nc.tensor.matmul(cB[:], trB_sb[:, 128:200], trB_sb[:],
                                     start=False, stop=True)
                    cA_sb = epi.tile([128, R], F16, name=f"cAs_{s}",
                                     tag="cAs", bufs=2)
                    cB_sb = epi.tile([72, R], F16, name=f"cBs_{s}",
                                     tag="cBs", bufs=2)
                    nc.vector.tensor_copy(cA_sb[:], cA[:])
                    nc.vector.tensor_copy(cB_sb[:], cB[:])
                    nc.sync.dma_start(out_d[s, 0:128, :], cA_sb[:])
                    nc.scalar.dma_start(out_d[s, 128:200, :], cB_sb[:])

    nc.compile()
    return nc


def _get_program(nA, nB):
    key = (nA, nB)
    if key not in _cached:
        _cached[key] = _build_program(nA, nB)
    return _cached[key]


def marshal_inputs(x, parc, mask):
    """Host-side prep: packed ROI-sorted fp16 x + tiny derived constants."""
    parc_eff = np.where(np.asarray(mask), np.asarray(parc), 0).reshape(V)
    lab = parc_eff.astype(np.int64) - 1          # -1 = dropped
    counts = np.bincount(parc_eff.astype(np.int64), minlength=R + 1)[1:]

    order = np.argsort(lab, kind="stable")
    nbg = int((lab < 0).sum())
    sorted_idx = order[nbg:]                     # kept pixels, ROI-ascending
    cA = int(counts[0:RA].sum())
    cB = int(counts[RA:R].sum())
    nA = (cA + 127) // 128
    nB = (cB + 127) // 128

    # Block B (ROIs 128..199) first, then block A.
    gB = np.concatenate([sorted_idx[cA:],
                         np.zeros(nB * 128 - cB, dtype=np.int64)])
    gA = np.concatenate([sorted_idx[:cA],
                         np.zeros(nA * 128 - cA, dtype=np.int64)])
    g = np.concatenate([gB, gA])                 # (nch*128,) gather indices
    labB = np.concatenate([lab[sorted_idx[cA:]] - RA,
                           np.full(nB * 128 - cB, -1, dtype=np.int64)])
    labA = np.concatenate([lab[sorted_idx[:cA]],
                           np.full(nA * 128 - cA, -1, dtype=np.int64)])
    nch = nA + nB
    labs = np.concatenate([labB, labA]).astype(np.float16)
    labs = labs.reshape(nch, 128).T.copy()       # (128, nch)

    iota = np.broadcast_to(np.arange(128, dtype=np.float16),
                           (128, 128)).copy()    # iota[p, c] = c
    i128 = np.eye(128, dtype=np.float16)
    i72 = np.zeros((128, 72), dtype=np.float16)
    i72[:72] = np.eye(72, dtype=np.float16)
    consts = np.concatenate([labs, iota, i128, i72], axis=1)  # (128, nch+328)

    # (N,1,T,H,W) fp32 -> packed (core, 128, nch, SPB*T) fp16
    x16 = np.asarray(x, dtype=np.float32).reshape(N, T, V).astype(np.float16)
    xg = x16[:, :, g]                            # (N, T, nch*128)
    xg = xg.reshape(NCORES, SPB, T, nch, 128)
    xs = np.ascontiguousarray(xg.transpose(0, 4, 3, 1, 2))  # (8,128,nch,2,T)
    xs = xs.reshape(NCORES, 128, nch, ROWS)

    in_maps = []
    for c in range(NCORES):
        in_maps.append({"x": xs[c], "consts": consts})
    return in_maps, nA, nB, counts


def kernel(x, parc, mask):
    in_maps, nA, nB, counts = marshal_inputs(x, parc, mask)
    nc = _get_program(nA, nB)
    res = run_bass_kernel_spmd(nc, in_maps, core_ids=list(range(NCORES)))
    # device emits the raw-sum Gram (fp16) + per-sample row sums; the
    # centering is a host-side rank-1 correction (C C^T = S S^T - m m^T/T
    # with m = row sums), and normalization a rank-1 scaling.
    G = np.stack([r["conn"] for r in res.results], axis=0)      # (8,2,200,200)
    G = G.reshape(N, R, R).astype(np.float64)
    ms = np.concatenate(
        [np.concatenate([r["msa"], r["msb"]], axis=0)[None]
         for r in res.results], axis=0)           # (8, 200, SPB)
    ms = ms.transpose(0, 2, 1).reshape(N, R).astype(np.float64)  # (16, 200)
    G -= ms[:, :, None] * ms[:, None, :] / T
    d = np.einsum('nrr->nr', G)                   # ||C_r||^2
    rinv = 1.0 / (np.sqrt(d) + counts[None, :] * EPS)
    conn = G * rinv[:, :, None] * rinv[:, None, :]
    row, col = np.triu_indices(R, k=1)
    return np.ascontiguousarray(conn[:, row, col]).astype(np.float32)


# revision 7
# speedup vs baseline: 1.1580x; 1.1580x over previous
"""Connectome kernel (segment-mean -> Pearson Gram) for 8 TRN2 NeuronCores.

Strategy (pure data parallel, 2 samples per core):
  - Host marshalling: fold mask into parcellation; DROP background /
    masked-out pixels (~50% of V) entirely; sort surviving pixels by ROI
    and pack them into 128-pixel chunks (block B = ROIs 128..199 FIRST,
    then block A = ROIs 0..127; each block padded to a chunk boundary
    with label -1 slots). x is gathered into this packed order, cast
    fp16, laid out [p, chunk, sample, t] per core so each SBUF partition
    reads one contiguous HBM run per chunk-tile. Wire traffic per core:
    ~18.3MB (vs 73.7MB for fp32 all-pixels).
  - Device: stream chunk-tiles on the two HWDGE rings, tiles assigned
    to rings greedily by cumulative bytes so both rings finish together;
    onehots per tile are built in batched DVE tensor_tensor ops
    (is_equal of broadcast iota vs broadcast labels); per chunk one PE
    matmul acc[r, row] += onehot.T @ x_chunk (fp16 operands, fp32 PSUM).
  - Centering cancels analytically: C C^T = S S^T - (1/T) m m^T with
    m = per-ROI row sums, so the device Grams the RAW sums S and ships
    the tiny row sums; the host applies the rank-1 correction and the
    1/norm scaling (norms from the corrected diagonal). The /counts
    ROI-mean scaling and the +eps normalizer cancel in the Pearson Gram.
  - Per block: one DVE cast of the PSUM sums to fp16 + one row-sum
    reduce, then PE transposes into [t, roi] tiles. Block B's work
    overlaps block A's stream; the tail is block A's cast + transposes,
    8 Gram matmuls (both samples packed per PSUM bank), two wide fp16
    conn DMAs.
  - Host: concat cores, rank-1 correct, normalize, upper triangle
    -> (16, 19900) fp32.
"""
import sys

sys.path.insert(0, "/opt/trn_rl_repo")

import numpy as np

import concourse.bass as bass
import concourse.tile as tile
from concourse import bacc, mybir
from concourse.bass_utils import run_bass_kernel_spmd

F32 = mybir.dt.float32
F16 = mybir.dt.float16

N, T, H, W = 16, 200, 144, 320
V = H * W                      # 46080
R = 200                        # ROIs
RA = 128                       # ROI block A width (ROIs 0..127)
RB = R - RA                    # ROI block B width (72; ROIs 128..199)
NCORES = 8
SPB = N // NCORES              # samples per core = 2
ROWS = SPB * T                 # 400
EPS = 1e-8


def _tile_sizes(nch):
    """DMA tile schedule: small first tiles to fill the pipe fast, 8s in
    steady state, small tapered tiles at the end so the PE drain after
    the last transfer is short."""
    sizes, left = [], nch
    while left and len(sizes) < 4:
        ct = min(4, left)
        sizes.append(ct)
        left -= ct
    while left >= 16:
        sizes.append(8)
        left -= 8
    if left > 8:
        sizes.append(left - 8)
        left = 8
    while left:
        ct = min(4, left)
        sizes.append(ct)
        left -= ct
    return sizes


_cached = {}


def _bc3(ap2, ins_pos, n):
    """Insert a broadcast (stride 0, count n) dim into a 2D AP."""
    layout = [list(d) for d in ap2.ap]
    layout.insert(ins_pos, [0, n])
    return bass.AP(ap2.tensor, ap2.offset, layout)


def _split_st(ap2):
    """View a [P, SPB*T] AP as [P, SPB, T] (split the free dim)."""
    layout = [list(d) for d in ap2.ap]
    assert layout[-1][0] == 1 and layout[-1][1] == SPB * T
    layout = layout[:-1] + [[T, SPB], [1, T]]
    return bass.AP(ap2.tensor, ap2.offset, layout)


def _build_program(nA, nB):
    nch = nA + nB
    nc = bacc.Bacc("TRN2", target_bir_lowering=False, debug=False)

    # consts packed into one DRAM tensor: cols [0:nch] labs, [nch:+128]
    # iota, [+128:+256] i128, [+256:+328] i72 (partitions 72:128 zero).
    CC = nch + 328
    x_d = nc.declare_dram_parameter("x", [128, nch, ROWS], F16, isOutput=False)
    cst_d = nc.declare_dram_parameter("consts", [128, CC], F16, isOutput=False)
    # conn2 cols: [0:200] G_s0 rois 0:128, [200:400] G_s1 rois 0:128,
    # [400:600] G_s0 rois 128:200 (parts 0:72), [600:800] G_s1 rois 128:200.
    out_d = nc.declare_dram_parameter("conn2", [128, 4 * R], F16, isOutput=True)
    msa_d = nc.declare_dram_parameter("msa", [RA, SPB], F32, isOutput=True)
    msb_d = nc.declare_dram_parameter("msb", [RB, SPB], F32, isOutput=True)

    tsizes = _tile_sizes(nch)
    # greedy byte-balanced ring assignment (sync starts with the consts)
    ring_bytes = {0: CC * 2.0, 1: 0.0}      # 0 = sync, 1 = scalar
    ring_of = []
    for ct in tsizes:
        r = 0 if ring_bytes[0] <= ring_bytes[1] else 1
        ring_of.append(r)
        ring_bytes[r] += ct * ROWS * 2.0

    with tile.TileContext(nc) as tc:
        with tc.tile_pool(name="consts", bufs=1) as consts, \
             tc.tile_pool(name="loads", bufs=3) as loads, \
             tc.tile_pool(name="ohp", bufs=1) as ohp, \
             tc.tile_pool(name="epi", bufs=1) as epi, \
             tc.tile_pool(name="psum", bufs=1, space="PSUM") as psum:

            cst_s = consts.tile([128, CC], F16)
            nc.sync.dma_start(cst_s[:], cst_d[:])
            labs_s = cst_s[:, 0:nch]
            iota_s = cst_s[:, nch:nch + 128]
            i128_s = cst_s[:, nch + 128:nch + 256]
            i72_s = cst_s[0:72, nch + 256:nch + 328]

            acc_a = psum.tile([RA, ROWS], F32, tag="acc_a", bufs=1)
            acc_b = psum.tile([RB, ROWS], F32, tag="acc_b", bufs=1)

            # PSUM tr tiles: [t-block, roi] transposed raw-sum rows.
            tr = {}
            for s in range(SPB):
                tr[("A", s)] = psum.tile([128, R], F16, tag="trA", bufs=2,
                                         name=f"trA_{s}")
                tr[("B", s)] = psum.tile([72, R], F16, tag="trB", bufs=2,
                                         name=f"trB_{s}")

            def finish_block(blk, acc, P, ms_d, ms_eng):
                """Raw-sum epilogue for one ROI block: cast the PSUM sums
                to fp16 (Gram/transpose operand) and ship per-sample row
                sums (host applies the rank-1 centering correction)."""
                S16 = epi.tile([P, ROWS], F16, tag=f"S16_{blk}")
                ms = epi.tile([P, SPB], F32, tag=f"ms_{blk}")
                nc.vector.tensor_copy(S16[:], acc[:])
                nc.vector.tensor_reduce(ms[:], _split_st(acc[:]),
                                        axis=mybir.AxisListType.X,
                                        op=mybir.AluOpType.add)
                ms_eng.dma_start(ms_d[:], ms[:])
                return S16

            S16_b = None
            with nc.named_scope("main"):
                ch0 = 0
                for ti, ct in enumerate(tsizes):
                    ld = loads.tile([128, ct, ROWS], F16, tag=f"ld{ct}",
                                    bufs=(12 if ct == 8 else 4),
                                    name=f"ld_{ti}")
                    eng = nc.sync if ring_of[ti] == 0 else nc.scalar
                    eng.dma_start(ld[:], x_d[:, ch0:ch0 + ct, :])

                    # batched per-tile onehot builds (DVE), one per block
                    # segment present in this tile
                    nb_i = max(0, min(nB, ch0 + ct) - ch0)       # B chunks
                    na_i = ct - nb_i                             # A chunks
                    ohB_t = ohA_t = None
                    if nb_i:
                        ohB_t = ohp.tile([128, nb_i, RB], F16,
                                         tag=f"ohB{nb_i}", bufs=4,
                                         name=f"ohB_{ti}")
                        nc.vector.tensor_tensor(
                            ohB_t[:], _bc3(iota_s[:, 0:RB], 1, nb_i),
                            _bc3(labs_s[:, ch0:ch0 + nb_i], 2, RB),
                            op=mybir.AluOpType.is_equal)
                    if na_i:
                        a0 = ch0 + nb_i
                        ohA_t = ohp.tile([128, na_i, RA], F16,
                                         tag=f"ohA{na_i}", bufs=4,
                                         name=f"ohA_{ti}")
                        nc.vector.tensor_tensor(
                            ohA_t[:], _bc3(iota_s[:, 0:RA], 1, na_i),
                            _bc3(labs_s[:, a0:a0 + na_i], 2, RA),
                            op=mybir.AluOpType.is_equal)

                    for j in range(ct):
                        cc = ch0 + j
                        if cc < nB:
                            acc, oh = acc_b, ohB_t[:, j, :]
                            start, stop = (cc == 0), (cc == nB - 1)
                        else:
                            acc, oh = acc_a, ohA_t[:, j - nb_i, :]
                            start, stop = (cc == nB), (cc == nch - 1)
                        nc.tensor.matmul(acc[:], oh, ld[:, j, :],
                                         start=start, stop=stop)
                    ch0 += ct

                    if ch0 - ct < nB <= ch0:
                        # block B complete: cast + row sums on DVE while
                        # block A still streams.
                        b_done_ti = ti
                        S16_b = finish_block("b", acc_b, RB, msb_d, nc.sync)
                    if S16_b is not None and ti == b_done_ti + 3:
                        # B-sourced transposes, emitted a few tiles later
                        # so the cast has finished and PE's FIFO never
                        # blocks on it.
                        for s in range(SPB):
                            nc.tensor.transpose(
                                tr[("A", s)][:, 128:200],
                                S16_b[:, s * T:s * T + 128], i72_s)
                            nc.tensor.transpose(
                                tr[("B", s)][:, 128:200],
                                S16_b[:, s * T + 128:s * T + 200], i72_s)

            with nc.named_scope("epilogue"):
                S16_a = finish_block("a", acc_a, RA, msa_d, nc.sync)
                tr_sb = {}
                for s in range(SPB):
                    nc.tensor.transpose(tr[("A", s)][:, 0:128],
                                        S16_a[:, s * T:s * T + 128], i128_s)
                    nc.tensor.transpose(tr[("B", s)][:, 0:128],
                                        S16_a[:, s * T + 128:s * T + 200],
                                        i128_s)
                    trA_sb = epi.tile([128, R], F16, name=f"trAs_{s}",
                                      tag="trAs", bufs=2)
                    trB_sb = epi.tile([72, R], F16, name=f"trBs_{s}",
                                      tag="trBs", bufs=2)
                    nc.vector.tensor_copy(trA_sb[:], tr[("A", s)][:])
                    nc.vector.tensor_copy(trB_sb[:], tr[("B", s)][:])
                    tr_sb[s] = (trA_sb, trB_sb)

                # Gram: conn = S_t.T @ S_t (contraction over t, fp16);
                # both samples packed into one PSUM bank per ROI-block so
                # each block ships as ONE wide cast + ONE wide DMA.
                cA = psum.tile([128, 2 * R], F32, tag="cA", bufs=1)
                cB = psum.tile([72, 2 * R], F32, tag="cB", bufs=1)
                connsb = epi.tile([128, 4 * R], F16, tag="connsb")
                for s in range(SPB):
                    trA_sb, trB_sb = tr_sb[s]
                    nc.tensor.matmul(cA[:, s * R:(s + 1) * R],
                                     trA_sb[:, 0:128], trA_sb[:],
                                     start=True, stop=False)
                    nc.tensor.matmul(cA[:, s * R:(s + 1) * R],
                                     trB_sb[:, 0:128], trB_sb[:],
                                     start=False, stop=True)
                nc.vector.tensor_copy(connsb[:, 0:2 * R], cA[:])
                nc.sync.dma_start(out_d[:, 0:2 * R], connsb[:, 0:2 * R])
                for s in range(SPB):
                    trA_sb, trB_sb = tr_sb[s]
                    nc.tensor.matmul(cB[:, s * R:(s + 1) * R],
                                     trA_sb[:, 128:200], trA_sb[:],
                                     start=True, stop=False)
                    nc.tensor.matmul(cB[:, s * R:(s + 1) * R],
                                     trB_sb[:, 128:200], trB_sb[:],
                                     start=False, stop=True)
                nc.vector.tensor_copy(connsb[0:72, 2 * R:4 * R], cB[:])
                nc.scalar.dma_start(out_d[0:72, 2 * R:4 * R],
                                    connsb[0:72, 2 * R:4 * R])

    nc.compile()
    return nc


def _get_program(nA, nB):
    key = (nA, nB)
    if key not in _cached:
        _cached[key] = _build_program(nA, nB)
    return _cached[key]


def marshal_inputs(x, parc, mask):
    """Host-side prep: packed ROI-sorted fp16 x + tiny derived constants."""
    parc_eff = np.where(np.asarray(mask), np.asarray(parc), 0).reshape(V)
    lab = parc_eff.astype(np.int64) - 1          # -1 = dropped
    counts = np.bincount(parc_eff.astype(np.int64), minlength=R + 1)[1:]

    order = np.argsort(lab, kind="stable")
    nbg = int((lab < 0).sum())
    sorted_idx = order[nbg:]                     # kept pixels, ROI-ascending
    cA = int(counts[0:RA].sum())
    cB = int(counts[RA:R].sum())
    nA = (cA + 127) // 128
    nB = (cB + 127) // 128

    # Block B (ROIs 128..199) first, then block A.
    gB = np.concatenate([sorted_idx[cA:],
                         np.zeros(nB * 128 - cB, dtype=np.int64)])
    gA = np.concatenate([sorted_idx[:cA],
                         np.zeros(nA * 128 - cA, dtype=np.int64)])
    g = np.concatenate([gB, gA])                 # (nch*128,) gather indices
    labB = np.concatenate([lab[sorted_idx[cA:]] - RA,
                           np.full(nB * 128 - cB, -1, dtype=np.int64)])
    labA = np.concatenate([lab[sorted_idx[:cA]],
                           np.full(nA * 128 - cA, -1, dtype=np.int64)])
    nch = nA + nB
    labs = np.concatenate([labB, labA]).astype(np.float16)
    labs = labs.reshape(nch, 128).T.copy()       # (128, nch)

    iota = np.broadcast_to(np.arange(128, dtype=np.float16),
                           (128, 128)).copy()    # iota[p, c] = c
    i128 = np.eye(128, dtype=np.float16)
    i72 = np.zeros((128, 72), dtype=np.float16)
    i72[:72] = np.eye(72, dtype=np.float16)
    consts = np.concatenate([labs, iota, i128, i72], axis=1)  # (128, nch+328)

    # (N,1,T,H,W) fp32 -> packed (core, 128, nch, SPB*T) fp16
    x16 = np.asarray(x, dtype=np.float32).reshape(N, T, V).astype(np.float16)
    xg = x16[:, :, g]                            # (N, T, nch*128)
    xg = xg.reshape(NCORES, SPB, T, nch, 128)
    xs = np.ascontiguousarray(xg.transpose(0, 4, 3, 1, 2))  # (8,128,nch,2,T)
    xs = xs.reshape(NCORES, 128, nch, ROWS)

    in_maps = []
    for c in range(NCORES):
        in_maps.append({"x": xs[c], "consts": consts})
    return in_maps, nA, nB, counts


def kernel(x, parc, mask):
    in_maps, nA, nB, counts = marshal_inputs(x, parc, mask)
    nc = _get_program(nA, nB)
    res = run_bass_kernel_spmd(nc, in_maps, core_ids=list(range(NCORES)))
    # device emits the raw-sum Gram (fp16) + per-sample row sums; the
    # centering is a host-side rank-1 correction (C C^T = S S^T - m m^T/T
    # with m = row sums), and normalization a rank-1 scaling.
    G = np.empty((NCORES, SPB, R, R), np.float64)
    for c, r in enumerate(res.results):
        c2 = r["conn2"].astype(np.float64)       # (128, 800)
        for s in range(SPB):
            G[c, s, 0:RA] = c2[:, s * R:(s + 1) * R]
            G[c, s, RA:R] = c2[0:72, (2 + s) * R:(3 + s) * R]
    G = G.reshape(N, R, R)
    ms = np.concatenate(
        [np.concatenate([r["msa"], r["msb"]], axis=0)[None]
         for r in res.results], axis=0)           # (8, 200, SPB)
    ms = ms.transpose(0, 2, 1).reshape(N, R).astype(np.float64)  # (16, 200)
    G -= ms[:, :, None] * ms[:, None, :] / T
    d = np.einsum('nrr->nr', G)                   # ||C_r||^2
    rinv = 1.0 / (np.sqrt(d) + counts[None, :] * EPS)
    conn = G * rinv[:, :, None] * rinv[:, None, :]
    row, col = np.triu_indices(R, k=1)
    return np.ascontiguousarray(conn[:, row, col]).astype(np.float32)


# revision 9
# speedup vs baseline: 1.2451x; 1.0753x over previous
"""Connectome kernel (segment-mean -> Pearson Gram) for 8 TRN2 NeuronCores.

Strategy (pure data parallel, 2 samples per core):
  - Host marshalling: fold mask into parcellation; DROP background /
    masked-out pixels (~50% of V) entirely; sort surviving pixels by ROI
    and pack them into 128-pixel chunks (block B = ROIs 128..199 FIRST,
    then block A = ROIs 0..127; each block padded to a chunk boundary
    with label -1 slots). x is gathered into this packed order, cast
    fp16, laid out [p, chunk, sample, t] per core so each SBUF partition
    reads one contiguous HBM run per chunk-tile. Wire traffic per core:
    ~18.3MB (vs 73.7MB for fp32 all-pixels).
  - Device: stream chunk-tiles on the two HWDGE rings, tiles assigned
    to rings greedily by cumulative bytes so both rings finish together;
    onehots per tile are built in batched DVE tensor_tensor ops
    (is_equal of broadcast iota vs broadcast labels); per chunk one PE
    matmul acc[r, row] += onehot.T @ x_chunk (fp16 operands, fp32 PSUM).
  - Centering cancels analytically: C C^T = S S^T - (1/T) m m^T with
    m = per-ROI row sums, so the device Grams the RAW sums S and ships
    the tiny row sums; the host applies the rank-1 correction and the
    1/norm scaling (norms from the corrected diagonal). The /counts
    ROI-mean scaling and the +eps normalizer cancel in the Pearson Gram.
  - Per block: one DVE cast of the PSUM sums to fp16 + one row-sum
    reduce, then PE transposes into [t, roi] tiles. Block B's work
    overlaps block A's stream; the tail is block A's cast + transposes,
    8 Gram matmuls (both samples packed per PSUM bank), two wide fp16
    conn DMAs.
  - Host: concat cores, rank-1 correct, normalize, upper triangle
    -> (16, 19900) fp32.
"""
import sys

sys.path.insert(0, "/opt/trn_rl_repo")

import numpy as np

import concourse.bass as bass
import concourse.tile as tile
from concourse import bacc, mybir
from concourse.bass_utils import run_bass_kernel_spmd

F32 = mybir.dt.float32
F16 = mybir.dt.float16

N, T, H, W = 16, 200, 144, 320
V = H * W                      # 46080
R = 200                        # ROIs
RA = 128                       # ROI block A width (ROIs 0..127)
RB = R - RA                    # ROI block B width (72; ROIs 128..199)
NCORES = 8
SPB = N // NCORES              # samples per core = 2
ROWS = SPB * T                 # 400
EPS = 1e-8


def _tile_sizes(nch):
    """DMA tile schedule: small first tiles to fill the pipe fast, 8s in
    steady state, small tapered tiles at the end so the PE drain after
    the last transfer is short."""
    sizes, left = [], nch
    while left and len(sizes) < 4:
        ct = min(4, left)
        sizes.append(ct)
        left -= ct
    while left >= 16:
        sizes.append(8)
        left -= 8
    if left > 8:
        sizes.append(left - 8)
        left = 8
    while left:
        ct = min(4, left)
        sizes.append(ct)
        left -= ct
    return sizes


_cached = {}


def _bc3(ap2, ins_pos, n):
    """Insert a broadcast (stride 0, count n) dim into a 2D AP."""
    layout = [list(d) for d in ap2.ap]
    layout.insert(ins_pos, [0, n])
    return bass.AP(ap2.tensor, ap2.offset, layout)


def _split_st(ap2):
    """View a [P, SPB*T] AP as [P, SPB, T] (split the free dim)."""
    layout = [list(d) for d in ap2.ap]
    assert layout[-1][0] == 1 and layout[-1][1] == SPB * T
    layout = layout[:-1] + [[T, SPB], [1, T]]
    return bass.AP(ap2.tensor, ap2.offset, layout)


def _build_program(nA, nB):
    nch = nA + nB
    nc = bacc.Bacc("TRN2", target_bir_lowering=False, debug=False)

    # consts packed into one DRAM tensor: cols [0:nch] labs, [nch:+128]
    # iota, [+128:+256] i128, [+256:+328] i72 (partitions 72:128 zero).
    CC = nch + 328
    x_d = nc.declare_dram_parameter("x", [128, nch, ROWS], F16, isOutput=False)
    cst_d = nc.declare_dram_parameter("consts", [128, CC], F16, isOutput=False)
    # conn2 cols: [0:200] G_s0 rois 0:128, [200:400] G_s1 rois 0:128,
    # [400:600] G_s0 rois 128:200 (parts 0:72), [600:800] G_s1 rois 128:200.
    out_d = nc.declare_dram_parameter("conn2", [128, 4 * R], F16, isOutput=True)
    msa_d = nc.declare_dram_parameter("msa", [RA, SPB], F32, isOutput=True)
    msb_d = nc.declare_dram_parameter("msb", [RB, SPB], F32, isOutput=True)

    tsizes = _tile_sizes(nch)
    # greedy byte-balanced ring assignment (sync starts with the consts)
    ring_bytes = {0: CC * 2.0, 1: 0.0}      # 0 = sync, 1 = scalar
    ring_of = []
    for ct in tsizes:
        r = 0 if ring_bytes[0] <= ring_bytes[1] else 1
        ring_of.append(r)
        ring_bytes[r] += ct * ROWS * 2.0

    with tile.TileContext(nc) as tc:
        with tc.tile_pool(name="consts", bufs=1) as consts, \
             tc.tile_pool(name="loads", bufs=3) as loads, \
             tc.tile_pool(name="ohp", bufs=1) as ohp, \
             tc.tile_pool(name="epi", bufs=1) as epi, \
             tc.tile_pool(name="psum", bufs=1, space="PSUM") as psum:

            cst_s = consts.tile([128, CC], F16)
            nc.sync.dma_start(cst_s[:], cst_d[:])
            labs_s = cst_s[:, 0:nch]
            iota_s = cst_s[:, nch:nch + 128]
            i128_s = cst_s[:, nch + 128:nch + 256]
            i72_s = cst_s[0:72, nch + 256:nch + 328]

            acc_a = psum.tile([RA, ROWS], F32, tag="acc_a", bufs=1)
            acc_b = psum.tile([RB, ROWS], F32, tag="acc_b", bufs=1)

            # PSUM tr tiles: [t-block, roi] transposed raw-sum rows.
            tr = {}
            for s in range(SPB):
                tr[("A", s)] = psum.tile([128, R], F16, tag="trA", bufs=2,
                                         name=f"trA_{s}")
                tr[("B", s)] = psum.tile([72, R], F16, tag="trB", bufs=2,
                                         name=f"trB_{s}")

            def finish_block(blk, acc, P, ms_d, ms_eng):
                """Raw-sum epilogue for one ROI block: cast the PSUM sums
                to fp16 (Gram/transpose operand) and ship per-sample row
                sums (host applies the rank-1 centering correction)."""
                S16 = epi.tile([P, ROWS], F16, tag=f"S16_{blk}")
                ms = epi.tile([P, SPB], F32, tag=f"ms_{blk}")
                nc.vector.tensor_copy(S16[:], acc[:])
                nc.vector.tensor_reduce(ms[:], _split_st(acc[:]),
                                        axis=mybir.AxisListType.X,
                                        op=mybir.AluOpType.add)
                ms_eng.dma_start(ms_d[:], ms[:])
                return S16

            S16_b = None
            with nc.named_scope("main"):
                ch0 = 0
                for ti, ct in enumerate(tsizes):
                    ld = loads.tile([128, ct, ROWS], F16, tag=f"ld{ct}",
                                    bufs=(16 if ct == 8 else 4),
                                    name=f"ld_{ti}")
                    eng = nc.sync if ring_of[ti] == 0 else nc.scalar
                    eng.dma_start(ld[:], x_d[:, ch0:ch0 + ct, :])

                    # batched per-tile onehot builds (DVE), one per block
                    # segment present in this tile
                    nb_i = max(0, min(nB, ch0 + ct) - ch0)       # B chunks
                    na_i = ct - nb_i                             # A chunks
                    ohB_t = ohA_t = None
                    if nb_i:
                        ohB_t = ohp.tile([128, nb_i, RB], F16,
                                         tag=f"ohB{nb_i}", bufs=4,
                                         name=f"ohB_{ti}")
                        nc.vector.tensor_tensor(
                            ohB_t[:], _bc3(iota_s[:, 0:RB], 1, nb_i),
                            _bc3(labs_s[:, ch0:ch0 + nb_i], 2, RB),
                            op=mybir.AluOpType.is_equal)
                    if na_i:
                        a0 = ch0 + nb_i
                        ohA_t = ohp.tile([128, na_i, RA], F16,
                                         tag=f"ohA{na_i}", bufs=4,
                                         name=f"ohA_{ti}")
                        nc.vector.tensor_tensor(
                            ohA_t[:], _bc3(iota_s[:, 0:RA], 1, na_i),
                            _bc3(labs_s[:, a0:a0 + na_i], 2, RA),
                            op=mybir.AluOpType.is_equal)

                    for j in range(ct):
                        cc = ch0 + j
                        if cc < nB:
                            acc, oh = acc_b, ohB_t[:, j, :]
                            start, stop = (cc == 0), (cc == nB - 1)
                        else:
                            acc, oh = acc_a, ohA_t[:, j - nb_i, :]
                            start, stop = (cc == nB), (cc == nch - 1)
                        nc.tensor.matmul(acc[:], oh, ld[:, j, :],
                                         start=start, stop=stop)
                    ch0 += ct

                    if ch0 - ct < nB <= ch0:
                        # block B complete: cast + row sums on DVE while
                        # block A still streams.
                        b_done_ti = ti
                        S16_b = finish_block("b", acc_b, RB, msb_d, nc.sync)
                    if S16_b is not None and ti == b_done_ti + 3:
                        # B-sourced transposes, emitted a few tiles later
                        # so the cast has finished and PE's FIFO never
                        # blocks on it.
                        for s in range(SPB):
                            nc.tensor.transpose(
                                tr[("A", s)][:, 128:200],
                                S16_b[:, s * T:s * T + 128], i72_s)
                            nc.tensor.transpose(
                                tr[("B", s)][:, 128:200],
                                S16_b[:, s * T + 128:s * T + 200], i72_s)

            with nc.named_scope("epilogue"):
                # block-A finish, casts split per sample so s0's transposes
                # start half a cast earlier.
                S16_a = epi.tile([RA, ROWS], F16, tag="S16_a")
                ms_a = epi.tile([RA, SPB], F32, tag="ms_a")
                tr_sb = {}
                for s in range(SPB):
                    nc.vector.tensor_copy(S16_a[:, s * T:(s + 1) * T],
                                          acc_a[:, s * T:(s + 1) * T])
                    nc.tensor.transpose(tr[("A", s)][:, 0:128],
                                        S16_a[:, s * T:s * T + 128], i128_s)
                    nc.tensor.transpose(tr[("B", s)][:, 0:128],
                                        S16_a[:, s * T + 128:s * T + 200],
                                        i128_s)
                    trA_sb = epi.tile([128, R], F16, name=f"trAs_{s}",
                                      tag="trAs", bufs=2)
                    trB_sb = epi.tile([72, R], F16, name=f"trBs_{s}",
                                      tag="trBs", bufs=2)
                    nc.vector.tensor_copy(trA_sb[:], tr[("A", s)][:])
                    nc.vector.tensor_copy(trB_sb[:], tr[("B", s)][:])
                    tr_sb[s] = (trA_sb, trB_sb)
                nc.vector.tensor_reduce(ms_a[:], _split_st(acc_a[:]),
                                        axis=mybir.AxisListType.X,
                                        op=mybir.AluOpType.add)
                nc.sync.dma_start(msa_d[:], ms_a[:])

                # Gram: conn = S_t.T @ S_t (contraction over t, fp16);
                # both samples packed into one PSUM bank per ROI-block;
                # per-sample casts + DMAs so outputs start flying while
                # the remaining Grams run. cB DMAs ship all 128
                # partitions (rows 72:128 are junk the host ignores) —
                # full-height transfers issue ~2x faster than 72-row ones.
                cA = psum.tile([128, 2 * R], F32, tag="cA", bufs=1)
                cB = psum.tile([72, 2 * R], F32, tag="cB", bufs=1)
                connsb = epi.tile([128, 4 * R], F16, tag="connsb")
                for s in range(SPB):
                    trA_sb, trB_sb = tr_sb[s]
                    nc.tensor.matmul(cA[:, s * R:(s + 1) * R],
                                     trA_sb[:, 0:128], trA_sb[:],
                                     start=True, stop=False)
                    nc.tensor.matmul(cA[:, s * R:(s + 1) * R],
                                     trB_sb[:, 0:128], trB_sb[:],
                                     start=False, stop=True)
                    nc.vector.tensor_copy(connsb[:, s * R:(s + 1) * R],
                                          cA[:, s * R:(s + 1) * R])
                    nc.sync.dma_start(out_d[:, s * R:(s + 1) * R],
                                      connsb[:, s * R:(s + 1) * R])
                for s in range(SPB):
                    trA_sb, trB_sb = tr_sb[s]
                    nc.tensor.matmul(cB[:, s * R:(s + 1) * R],
                                     trA_sb[:, 128:200], trA_sb[:],
                                     start=True, stop=False)
                    nc.tensor.matmul(cB[:, s * R:(s + 1) * R],
                                     trB_sb[:, 128:200], trB_sb[:],
                                     start=False, stop=True)
                    nc.vector.tensor_copy(connsb[0:72, (2 + s) * R:(3 + s) * R],
                                          cB[:, s * R:(s + 1) * R])
                    nc.scalar.dma_start(out_d[:, (2 + s) * R:(3 + s) * R],
                                        connsb[:, (2 + s) * R:(3 + s) * R])

    nc.compile()
    return nc


def _get_program(nA, nB):
    key = (nA, nB)
    if key not in _cached:
        _cached[key] = _build_program(nA, nB)
    return _cached[key]


def marshal_inputs(x, parc, mask):
    """Host-side prep: packed ROI-sorted fp16 x + tiny derived constants."""
    parc_eff = np.where(np.asarray(mask), np.asarray(parc), 0).reshape(V)
    lab = parc_eff.astype(np.int64) - 1          # -1 = dropped
    counts = np.bincount(parc_eff.astype(np.int64), minlength=R + 1)[1:]

    order = np.argsort(lab, kind="stable")
    nbg = int((lab < 0).sum())
    sorted_idx = order[nbg:]                     # kept pixels, ROI-ascending
    cA = int(counts[0:RA].sum())
    cB = int(counts[RA:R].sum())
    nA = (cA + 127) // 128
    nB = (cB + 127) // 128

    # Block B (ROIs 128..199) first, then block A.
    gB = np.concatenate([sorted_idx[cA:],
                         np.zeros(nB * 128 - cB, dtype=np.int64)])
    gA = np.concatenate([sorted_idx[:cA],
                         np.zeros(nA * 128 - cA, dtype=np.int64)])
    g = np.concatenate([gB, gA])                 # (nch*128,) gather indices
    labB = np.concatenate([lab[sorted_idx[cA:]] - RA,
                           np.full(nB * 128 - cB, -1, dtype=np.int64)])
    labA = np.concatenate([lab[sorted_idx[:cA]],
                           np.full(nA * 128 - cA, -1, dtype=np.int64)])
    nch = nA + nB
    labs = np.concatenate([labB, labA]).astype(np.float16)
    labs = labs.reshape(nch, 128).T.copy()       # (128, nch)

    iota = np.broadcast_to(np.arange(128, dtype=np.float16),
                           (128, 128)).copy()    # iota[p, c] = c
    i128 = np.eye(128, dtype=np.float16)
    i72 = np.zeros((128, 72), dtype=np.float16)
    i72[:72] = np.eye(72, dtype=np.float16)
    consts = np.concatenate([labs, iota, i128, i72], axis=1)  # (128, nch+328)

    # (N,1,T,H,W) fp32 -> packed (core, 128, nch, SPB*T) fp16
    x16 = np.asarray(x, dtype=np.float32).reshape(N, T, V).astype(np.float16)
    xg = x16[:, :, g]                            # (N, T, nch*128)
    xg = xg.reshape(NCORES, SPB, T, nch, 128)
    xs = np.ascontiguousarray(xg.transpose(0, 4, 3, 1, 2))  # (8,128,nch,2,T)
    xs = xs.reshape(NCORES, 128, nch, ROWS)

    in_maps = []
    for c in range(NCORES):
        in_maps.append({"x": xs[c], "consts": consts})
    return in_maps, nA, nB, counts


def kernel(x, parc, mask):
    in_maps, nA, nB, counts = marshal_inputs(x, parc, mask)
    nc = _get_program(nA, nB)
    res = run_bass_kernel_spmd(nc, in_maps, core_ids=list(range(NCORES)))
    # device emits the raw-sum Gram (fp16) + per-sample row sums; the
    # centering is a host-side rank-1 correction (C C^T = S S^T - m m^T/T
    # with m = row sums), and normalization a rank-1 scaling.
    G = np.empty((NCORES, SPB, R, R), np.float64)
    for c, r in enumerate(res.results):
        c2 = r["conn2"].astype(np.float64)       # (128, 800)
        for s in range(SPB):
            G[c, s, 0:RA] = c2[:, s * R:(s + 1) * R]
            G[c, s, RA:R] = c2[0:72, (2 + s) * R:(3 + s) * R]
    G = G.reshape(N, R, R)
    ms = np.concatenate(
        [np.concatenate([r["msa"], r["msb"]], axis=0)[None]
         for r in res.results], axis=0)           # (8, 200, SPB)
    ms = ms.transpose(0, 2, 1).reshape(N, R).astype(np.float64)  # (16, 200)
    G -= ms[:, :, None] * ms[:, None, :] / T
    d = np.einsum('nrr->nr', G)                   # ||C_r||^2
    rinv = 1.0 / (np.sqrt(d) + counts[None, :] * EPS)
    conn = G * rinv[:, :, None] * rinv[:, None, :]
    row, col = np.triu_indices(R, k=1)
    return np.ascontiguousarray(conn[:, row, col]).astype(np.float32)
